# revision 1
# baseline (speedup 1.0000x reference)
"""DifferentialAttentionBlock on 8 NeuronCores.

Sharding: DP on batch (cores 0-3 = batch 0, 4-7 = batch 1) x TP on heads
(4 heads per core) for everything through attention; then an 8-rank
AllGather of bf16 attnT shards and a 128-column Wo shard per core over
both batches (keeps the program free of per-core offsets).

Per-core dataflow (transposed-activation layout):
  qT/kT/vT (host-transposed) -> projections q1T/q2T/k1T/k2T [128c, S]
  (fp32r) and vv [S, 256] (+ones col, bf16) -> per-head transposed
  scores (32x128 row-tiled PE, fp32r) -> exp (ACT, scale=1/8, max-free)
  -> bf16 A@V in outT form (vv stationary; fused colsum row) ->
  reciprocal + gpsimd partition-broadcast normalize + lambda combine
  straight into attnT bf16 -> chunked 8-rank AllGather -> Wo col-shard
  bf16 matmul -> out [2*S, 128] per core.  Output bias bo on host.
"""

import math
import numpy as np

B, S, D = 2, 1024, 1024
H = 16
DH = 32          # q/k half head dim
DK = 64          # v head dim
HPC = 4          # heads per core
CPB = 4          # cores per batch (TP group size)
NCORES = 8
LAMBDA_INIT = 0.8 - 0.6 * math.exp(-0.3 * (1 - 1))
NSK = S // 128   # 8 s_k tiles
CHW = 512        # sq chunk width
NCH = S // CHW   # 2 chunks
RG8 = [list(range(8))]

PROFILE = False
LAST_EXEC_NS = None
LAST_RESULTS = None

_cache = {}


def _try_install_ntff_hook():
    try:
        import sys, types
        import antenv
        try:
            import antenv.axon_hooks  # noqa: F401
            return
        except ImportError:
            pass
        mod = types.ModuleType("antenv.axon_hooks")
        mod._hook = None
        mod.set_axon_ntff_profile_hook = lambda h: setattr(mod, "_hook", h)
        mod.get_axon_ntff_profile_hook = lambda: mod._hook
        sys.modules["antenv.axon_hooks"] = mod
        antenv.axon_hooks = mod
        from trn_agent_boot.trn_boot import _ntff_profile_via_ctypes
        mod._hook = _ntff_profile_via_ctypes('/opt/axon/libaxon_pjrt.so')
    except Exception:
        pass


def _build(causal: bool):
    import concourse.bacc as bacc
    import concourse.mybir as mybir
    import concourse.tile as tile
    from concourse.tile_rust import add_dep_helper

    dt = mybir.dt
    f32, f32r, bf16 = dt.float32, dt.float32r, dt.bfloat16
    AF = mybir.ActivationFunctionType
    OP = mybir.AluOpType

    nc = bacc.Bacc("TRN2", target_bir_lowering=False, debug=False,
                   num_devices=NCORES)

    def inp(name, shape, d=f32):
        return nc.dram_tensor(name, shape, d, kind="ExternalInput")

    qT = inp("qT", [D, S], bf16)
    kT = inp("kT", [D, S], bf16)
    vT = inp("vT", [D, S], bf16)
    Wq1 = inp("Wq1", [D, 128], bf16);  Wq2 = inp("Wq2", [D, 128], bf16)
    Wk1 = inp("Wk1", [D, 128], bf16);  Wk2 = inp("Wk2", [D, 128], bf16)
    Wv = inp("Wv", [D, 256], bf16)
    Wob = inp("Wob", [D, 128], bf16)          # my 128 output columns
    bq1 = inp("bq1", [128, 1]);  bq2 = inp("bq2", [128, 1])
    bk1 = inp("bk1", [128, 1]);  bk2 = inp("bk2", [128, 1])
    bv = inp("bv", [1, 256], bf16)
    ones_in = inp("ones1", [1, 128], bf16)
    triu = inp("triu", [128, 128], bf16)
    lamv = inp("lamv", [128, 1])
    maskT = None if causal else inp("maskT", [S, S])
    out_ext = nc.dram_tensor("out", [128, B * S], f32, kind="ExternalOutput")

    with tile.TileContext(nc) as tc:
        with (
            tc.tile_pool(name="const", bufs=1) as cpool,
            tc.tile_pool(name="wts", bufs=1) as wpool,
            tc.tile_pool(name="proj", bufs=1) as ppool,
            tc.tile_pool(name="acts", bufs=2) as apool,
            tc.tile_pool(name="edata", bufs=2) as epool,
            tc.tile_pool(name="small", bufs=2) as spool,
            tc.tile_pool(name="outs", bufs=2) as opool,
            tc.tile_pool(name="dram", bufs=1, space="DRAM") as dpool,
        ):
            # tiny AllGather issued first: absorbs cross-core launch skew
            # while the load phase runs, so real collectives see synced peers
            dummy_in = dpool.tile([1, 16], bf16, name="dummy_in")
            dummy_out = dpool.tile([8, 16], bf16, name="dummy_out")
            nc.gpsimd.dma_start(dummy_in[:], triu[0:1, 0:16])
            nc.gpsimd.collective_compute(
                "AllGather", mybir.AluOpType.bypass, replica_groups=RG8,
                ins=[dummy_in.opt()], outs=[dummy_out.opt()])
            # ---- constants + q/k projection weights first ----
            triu_sb = cpool.tile([128, 128], bf16, tag="triu")
            nc.sync.dma_start(triu_sb[:], triu[:, :])
            lam_sb = cpool.tile([128, 1], f32, tag="lamv")
            nc.sync.dma_start(lam_sb[:], lamv[:, :])
            ones1 = cpool.tile([1, 128], bf16, tag="ones1")
            nc.sync.dma_start(ones1[:], ones_in[:, :])
            bsb = {}
            for name, t in (("bq1", bq1), ("bq2", bq2), ("bk1", bk1),
                            ("bk2", bk2)):
                bsb[name] = cpool.tile([128, 1], f32, tag=name, name=name)
                nc.sync.dma_start(bsb[name][:], t[:, :])
            bv_sb = cpool.tile([1, 256], bf16, tag="bv")
            nc.sync.dma_start(bv_sb[:], bv[:, :])

            wsb = {}
            for name, t in (("Wq1", Wq1), ("Wq2", Wq2),
                            ("Wk1", Wk1), ("Wk2", Wk2)):
                wsb[name] = wpool.tile([128, 8 * 128], bf16, tag=name,
                                       name=name)
                # one 3-D DMA: dram [8, 128, 128] -> sbuf [128, (8, 128)]
                nc.sync.dma_start(
                    wsb[name][:].rearrange("p (d c) -> p d c", d=8),
                    t.rearrange("(d p) c -> p d c", p=128))

            vtiles = []
            # kpad tiles zeroed early (DVE idle during load phase)
            kpad = {}
            for m_ in range(2):
                for h_ in range(4):
                    t_ = ppool.tile([128, S], bf16, tag=f"kp{m_}{h_}",
                                    name=f"kp{m_}{h_}")
                    nc.vector.memset(t_[:], 0.0)
                    kpad[(m_, h_)] = t_
            # ---- phase B: q then k projections (pipelined drains) ----
            with tc.tile_pool(name="psBq", bufs=1, space="PSUM") as psBq:
                pq1 = psBq.tile([128, S], f32, tag="q1")
                pq2 = psBq.tile([128, S], f32, tag="q2")
                qblk = []
                for g in range(2):
                    qb = apool.tile([128, 4 * S], bf16, tag="qTd",
                                    name=f"qblk{g}")
                    nc.sync.dma_start(
                        qb[:].rearrange("p (d x) -> p d x", d=4),
                        qT[g * 512:(g + 1) * 512, :]
                        .rearrange("(d p) x -> p d x", p=128))
                    qblk.append(qb)
                for d in range(8):
                    qTd = qblk[d // 4][:, (d % 4) * S:(d % 4 + 1) * S]
                    for ps, wname in ((pq1, "Wq1"), (pq2, "Wq2")):
                        lhsT = wsb[wname][:, d * 128:(d + 1) * 128]
                        for half in range(2):
                            nc.tensor.matmul(
                                ps[:, half * 512:(half + 1) * 512], lhsT,
                                qTd[:, half * 512:(half + 1) * 512],
                                start=(d == 0), stop=(d == 7))
                q1T = ppool.tile([128, S], bf16, tag="q1T")
                q2T = ppool.tile([128, S], bf16, tag="q2T")
                nc.scalar.activation(q1T[:], pq1[:], AF.Identity,
                                     bias=bsb["bq1"][:])
                nc.vector.tensor_scalar(q2T[:], pq2[:], bsb["bq2"][:],
                                        None, OP.add)
            with tc.tile_pool(name="psBk", bufs=1, space="PSUM") as psBk:
                pk1 = psBk.tile([128, S], f32, tag="k1")
                pk2 = psBk.tile([128, S], f32, tag="k2")
                kblk = []
                for g in range(2):
                    kb = apool.tile([128, 4 * S], bf16, tag="kTd",
                                    name=f"kblk{g}")
                    nc.scalar.dma_start(
                        kb[:].rearrange("p (d x) -> p d x", d=4),
                        kT[g * 512:(g + 1) * 512, :]
                        .rearrange("(d p) x -> p d x", p=128))
                    kblk.append(kb)
                    vb = apool.tile([128, 4 * S], bf16, tag="vTd",
                                    name=f"vblk{g}")
                    nc.sync.dma_start(
                        vb[:].rearrange("p (d x) -> p d x", d=4),
                        vT[g * 512:(g + 1) * 512, :]
                        .rearrange("(d p) x -> p d x", p=128))
                    vtiles.append(vb)
                for d in range(8):
                    kTd = kblk[d // 4][:, (d % 4) * S:(d % 4 + 1) * S]
                    for ps, wname in ((pk1, "Wk1"), (pk2, "Wk2")):
                        lhsT = wsb[wname][:, d * 128:(d + 1) * 128]
                        for half in range(2):
                            nc.tensor.matmul(
                                ps[:, half * 512:(half + 1) * 512], lhsT,
                                kTd[:, half * 512:(half + 1) * 512],
                                start=(d == 0), stop=(d == 7))
                # k projections land in the zero-padded per-head tiles so
                # the score matmuls run as plain full-array K=128 bf16 GEMMs
                for m, pk, bn in ((0, pk1, "bk1"), (1, pk2, "bk2")):
                    for h in range(4):
                        t = kpad[(m, h)]
                        sl = slice(32 * h, 32 * h + 32)
                        if m == 0:
                            nc.scalar.activation(t[sl, :], pk[sl, :],
                                                 AF.Identity,
                                                 bias=bsb[bn][sl, :])
                        else:
                            nc.vector.tensor_scalar(t[sl, :], pk[sl, :],
                                                    bsb[bn][sl, :],
                                                    None, OP.add)
            # ---- phase C: vv projection (natural layout) + ones cols ----
            wv_sb = wpool.tile([128, 8 * 256], bf16, tag="Wv")
            nc.gpsimd.dma_start(wv_sb[:].rearrange("p (d c) -> p d c", d=8),
                                Wv.rearrange("(d p) c -> p d c", p=128))
            # vvo layout: [128, 8*260] bf16; block i: 4 heads x (vv x64, 1)
            vvo = ppool.tile([128, 8 * 260], bf16, tag="vvo")
            with tc.tile_pool(name="psC", bufs=1, space="PSUM") as psC:
                pvv = [psC.tile([128, 256], f32, tag=f"vv{i}", name=f"vv{i}")
                       for i in range(8)]
                for d in range(8):
                    vTd = vtiles[d // 4][:, (d % 4) * S:(d % 4 + 1) * S]
                    for i in range(8):
                        nc.tensor.matmul(
                            pvv[i][:], vTd[:, i * 128:(i + 1) * 128],
                            wv_sb[:, d * 256:(d + 1) * 256],
                            start=(d == 0), stop=False)
                for i in range(8):
                    nc.tensor.matmul(pvv[i][:], ones1[:], bv_sb[:],
                                     start=False, stop=True)
                    blk = vvo[:, i * 260:(i + 1) * 260]
                    blk3 = blk.rearrange("p (h c) -> p h c", c=65)
                    nc.vector.tensor_copy(
                        blk3[:, :, 0:64],
                        pvv[i].rearrange("p (h c) -> p h c", c=64))
                    nc.vector.memset(blk3[:, :, 64:65], 1.0)

            # Wo col-shard (needed only in phase F; load early, off crit path)
            wo_sb = wpool.tile([128, 8 * 128], bf16, tag="Wob")
            nc.gpsimd.dma_start(wo_sb[:].rearrange("p (d c) -> p d c", d=8),
                                Wob.rearrange("(d p) c -> p d c", p=128))

            # ---- phase D: attention; attnT assembled directly ----
            aT_sb = [ppool.tile([128, S], bf16, tag=f"aT{kk}", name=f"aT{kk}")
                     for kk in range(2)]
            qproj = (q1T, q2T)
            last_sub = [None]
            bounce_t = dpool.tile([256, S], bf16, name="bounce_t")
            ag_full = dpool.tile([NCORES * 256, S], bf16, name="ag_full")

            def wo_all():
                """Wo matmuls over the gathered attnT; output stored
                transposed as [128 my-cols, B*S] (host un-transposes)."""
                with tc.tile_pool(name="psW", bufs=2,
                                  space="PSUM") as psW:
                    for half in range(2):
                        myt = opool.tile([128, 8 * S], bf16, tag="myt",
                                         name=f"myt{half}", bufs=2)
                        mydma = nc.sync.dma_start(
                            myt[:].rearrange("p (k x) -> p k x", k=8),
                            ag_full[half * 1024:(half + 1) * 1024, :]
                            .rearrange("(k p) x -> p k x", p=128))
                        if last_sub[0] is not None:
                            add_dep_helper(mydma.ins, last_sub[0].ins,
                                           reason="wo after normalize")
                        # half = batch; out.T[mycol, sq] = Wo_shard.T @ attnT
                        for nch in range(2):
                            wps = psW.tile([128, 512], f32, tag="wo",
                                           name=f"wo{half}{nch}")
                            for k in range(8):
                                nc.tensor.matmul(
                                    wps[:],
                                    wo_sb[:, 128 * k:128 * (k + 1)],
                                    myt[:, S * k + 512 * nch:
                                        S * k + 512 * (nch + 1)],
                                    start=(k == 0), stop=(k == 7))
                            osb = opool.tile([128, 512], f32, tag="osb",
                                             name=f"osb{half}{nch}")
                            if nch % 2 == 0:
                                nc.scalar.copy(osb[:], wps[:])
                            else:
                                nc.vector.tensor_copy(osb[:], wps[:])
                            col = half * S + 512 * nch
                            nc.sync.dma_start(out_ext[:, col:col + 512],
                                              osb[:])

            with (
                tc.tile_pool(name="psS", bufs=2, space="PSUM") as psS,
                tc.tile_pool(name="psAV", bufs=2, space="PSUM") as psAV,
                tc.tile_pool(name="mloc", bufs=2) as mpool,
            ):
                tfs_prev = {}
                ocps = {}
                rdir = {}
                last_sub[0] = None
                for c in range(NCH):
                    cs = c * CHW
                    ilist = list(range(min(NSK, (cs + CHW) // 128))) if causal \
                        else list(range(NSK))
                    if not causal:
                        mT = {}
                        for i in ilist:
                            mT[i] = mpool.tile([128, CHW], f32, tag=f"mT{i%2}",
                                               name=f"mT{c}{i}")
                            nc.sync.dma_start(
                                mT[i][:],
                                maskT[i * 128:(i + 1) * 128, cs:cs + CHW])
                    crows = [spool.tile([1, 4 * CHW], f32, tag=f"crows{m_}",
                                        name=f"crows{m_}c{c}", bufs=1)
                             for m_ in range(2)]
                    cdr = [dpool.tile([4 * CHW], f32, name=f"cdr{m_}c{c}")
                           for m_ in range(2)]
                    rdr = [dpool.tile([4 * CHW], f32, name=f"rdr{m_}c{c}")
                           for m_ in range(2)]
                    for m in range(2):
                        for h in range(4):
                            etiles = []
                            for i in ilist:
                                lo = max(cs, 128 * i) if causal else cs
                                n = cs + CHW - lo
                                ps = psS.tile([128, CHW], f32, tag=f"s{h%2}",
                                              name=f"s{c}{i}{m}{h}",
                                              bufs=2 if h % 2 == 0 else 1)
                                nc.tensor.matmul(
                                    ps[:, 0:n],
                                    kpad[(m, h)][:, i * 128:(i + 1) * 128],
                                    qproj[m][:, lo:cs + CHW],
                                    start=True, stop=True)
                                if not causal:
                                    nc.vector.tensor_tensor(
                                        ps[:, 0:n], ps[:, 0:n],
                                        mT[i][:, 0:n], OP.add)
                                e = epool.tile([128, CHW], bf16,
                                               tag=f"e{i}",
                                               name=f"e{m}h{h}i{i}c{c}")
                                etiles.append((i, e))
                                nc.scalar.activation(
                                    e[:, lo - cs:CHW], ps[:, 0:n], AF.Exp,
                                    scale=0.125)
                                if causal and 128 * i >= cs:
                                    off = 128 * i - cs
                                    nc.vector.tensor_tensor(
                                        e[:, off:off + 128],
                                        e[:, off:off + 128], triu_sb[:],
                                        OP.mult)
                            # A@V outT form: o[0:64] = vv_h.T @ e, o[64]=colsum
                            o = psAV.tile([128, CHW], f32, tag=f"o{m}",
                                          name=f"o{m}h{h}c{c}")
                            for x, (i, e) in enumerate(etiles):
                                lo = max(cs, 128 * i) if causal else cs
                                nc.tensor.matmul(
                                    o[0:65, lo - cs:CHW],
                                    vvo[:, 260 * i + 65 * h:
                                        260 * i + 65 * h + 65],
                                    e[:, lo - cs:CHW],
                                    start=(x == 0), stop=(x == len(etiles) - 1))
                            # copy out of PSUM right away (frees the bank)
                            ocp = spool.tile([64, CHW], f32, tag=f"ocp{m}{h}",
                                             name=f"ocp{m}{h}c{c}", bufs=1)
                            ocps[(m, h)] = ocp
                            nc.vector.tensor_copy(ocp[:], o[0:64, 0:CHW])
                            if c == NCH - 1 and m == 1:
                                rdir[(m, h)] = rd = crows[m][
                                    0:1, CHW * h:CHW * (h + 1)]
                                nc.vector.reciprocal(rd, o[64:65, 0:CHW])
                                nc.vector.tensor_scalar(
                                    rd, rd, lam_sb[0:1, 0:1], None, OP.mult)
                            else:
                                nc.vector.tensor_copy(
                                    crows[m][0:1, CHW * h:CHW * (h + 1)],
                                    o[64:65, 0:CHW])
                        if c == NCH - 1 and m == 1:
                            for h in range(4):
                                rb = spool.tile([64, CHW], f32, tag=f"rb{m}",
                                                name=f"rbd{m}h{h}c{c}")
                                nc.gpsimd.partition_broadcast(
                                    rb[:], rdir[(m, h)])
                                tf = spool.tile([64, CHW], f32,
                                                tag=f"tf{m}h{h}" if m == 0
                                                else f"tf{m}",
                                                name=f"tfd{m}h{h}c{c}",
                                                bufs=1 if m == 0 else 2)
                                nc.vector.tensor_tensor(
                                    tf[:], ocps[(m, h)][:], rb[:], OP.mult)
                                if m == 1:
                                    dst = aT_sb[h // 2][
                                        64 * (h % 2):64 * (h % 2) + 64,
                                        cs:cs + CHW]
                                    last_sub[0] = nc.vector.tensor_tensor(
                                        dst, tfs_prev[h], tf[:], OP.subtract)
                                else:
                                    tfs_prev[h] = tf
                            continue
                        # batched 1/colsum for all 4 heads of this m:
                        # row [1,4*CHW] -> dram -> [128, 4*CHW/128] -> recip
                        nc.gpsimd.dma_start(cdr[m][:], crows[m][0:1, :])
                        cT = spool.tile([128, 4 * CHW // 128], f32,
                                        tag=f"cT{m}", name=f"cT{m}c{c}")
                        nc.gpsimd.dma_start(
                            cT[:], cdr[m].rearrange("(p x) -> p x", p=128))
                        nc.vector.reciprocal(cT[:], cT[:])
                        if m == 1:
                            nc.vector.tensor_scalar(cT[:], cT[:],
                                                    lam_sb[:, 0:1], None,
                                                    OP.mult)
                        nc.gpsimd.dma_start(rdr[m][:].rearrange(
                            "(p x) -> p x", p=128), cT[:])
                        rr = crows[m]
                        nc.gpsimd.dma_start(rr[:], rdr[m][:])
                        for h in range(4):
                            rb = spool.tile([64, CHW], f32, tag=f"rb{m}",
                                            name=f"rb{m}h{h}c{c}")
                            nc.gpsimd.partition_broadcast(
                                rb[:], rr[0:1, CHW * h:CHW * (h + 1)])
                            tf = spool.tile([64, CHW], f32,
                                            tag=f"tf{m}h{h}" if m == 0
                                            else f"tf{m}",
                                            name=f"tf{m}h{h}c{c}",
                                            bufs=1 if m == 0 else 2)
                            nc.vector.tensor_tensor(tf[:], ocps[(m, h)][:],
                                                    rb[:], OP.mult)
                            if m == 1:
                                dst = aT_sb[h // 2][
                                    64 * (h % 2):64 * (h % 2) + 64,
                                    cs:cs + CHW]
                                last_sub[0] = nc.vector.tensor_tensor(
                                    dst, tfs_prev[h], tf[:], OP.subtract)
                            else:
                                tfs_prev[h] = tf
                # ship attnT; single AllGather across all 8 cores
                for kk in range(2):
                    nc.sync.dma_start(bounce_t[128 * kk:128 * (kk + 1), :],
                                      aT_sb[kk][:])
                nc.gpsimd.collective_compute(
                    "AllGather", mybir.AluOpType.bypass, replica_groups=RG8,
                    ins=[bounce_t.opt()], outs=[ag_full.opt()])
            wo_all()

    nc.compile()
    return nc


def kernel(**inputs):
    global LAST_EXEC_NS
    import ml_dtypes

    q = np.asarray(inputs["q"], dtype=np.float32)
    k = np.asarray(inputs["k"], dtype=np.float32)
    v = np.asarray(inputs["v"], dtype=np.float32)
    mask = np.asarray(inputs["mask"])
    f32 = np.float32
    Wq1f = np.asarray(inputs["Wq1"], f32); Wq2f = np.asarray(inputs["Wq2"], f32)
    Wk1f = np.asarray(inputs["Wk1"], f32); Wk2f = np.asarray(inputs["Wk2"], f32)
    Wvf = np.asarray(inputs["Wv"], f32);   Wof = np.asarray(inputs["Wo"], f32)
    bq1f = np.asarray(inputs["bq1"], f32); bq2f = np.asarray(inputs["bq2"], f32)
    bk1f = np.asarray(inputs["bk1"], f32); bk2f = np.asarray(inputs["bk2"], f32)
    bvf = np.asarray(inputs["bv"], f32);   bof = np.asarray(inputs["bo"], f32)
    lam = float(np.exp(float(inputs["lq1"][0]) * float(inputs["lk1"][0]))
                - np.exp(float(inputs["lq2"][0]) * float(inputs["lk2"][0]))
                + LAMBDA_INIT)

    mk = (mask.reshape(B, S, S) != 0)
    causal = bool((mk == np.tril(np.ones((S, S), bool))[None]).all())

    key = "causal" if causal else "general"
    if key not in _cache:
        _cache[key] = _build(causal)
    nc = _cache[key]

    bfl = ml_dtypes.bfloat16
    qT = [np.ascontiguousarray(q[b].T).astype(bfl) for b in range(B)]
    kTl = [np.ascontiguousarray(k[b].T).astype(bfl) for b in range(B)]
    vTl = [np.ascontiguousarray(v[b].T).astype(bfl) for b in range(B)]
    Wob = Wof.astype(ml_dtypes.bfloat16)
    triu = np.triu(np.ones((128, 128))).astype(ml_dtypes.bfloat16)
    lamv = np.full((128, 1), lam, f32)
    maskTs = None
    if not causal:
        maskTs = [np.ascontiguousarray(
            np.where(mk[b], np.float32(0), np.float32(-1e9)).T)
            for b in range(B)]

    in_maps = []
    for c in range(NCORES):
        b, g = divmod(c, CPB)
        im = dict(
            qT=qT[b], kT=kTl[b], vT=vTl[b],
            Wq1=np.ascontiguousarray(Wq1f[:, 128 * g:128 * (g + 1)]).astype(bfl),
            Wq2=np.ascontiguousarray(Wq2f[:, 128 * g:128 * (g + 1)]).astype(bfl),
            Wk1=np.ascontiguousarray(Wk1f[:, 128 * g:128 * (g + 1)]).astype(bfl),
            Wk2=np.ascontiguousarray(Wk2f[:, 128 * g:128 * (g + 1)]).astype(bfl),
            Wv=np.ascontiguousarray(Wvf[:, 256 * g:256 * (g + 1)]).astype(bfl),
            Wob=np.ascontiguousarray(Wob[:, 128 * c:128 * (c + 1)]),
            bq1=np.ascontiguousarray(bq1f[128 * g:128 * (g + 1)]).reshape(128, 1),
            bq2=np.ascontiguousarray(bq2f[128 * g:128 * (g + 1)]).reshape(128, 1),
            bk1=np.ascontiguousarray(bk1f[128 * g:128 * (g + 1)]).reshape(128, 1),
            bk2=np.ascontiguousarray(bk2f[128 * g:128 * (g + 1)]).reshape(128, 1),
            bv=np.ascontiguousarray(bvf[256 * g:256 * (g + 1)]).reshape(1, 256).astype(bfl),
            triu=triu, lamv=lamv,
            ones1=np.ones((1, 128), bfl),
        )
        if not causal:
            im["maskT"] = maskTs[b]
        in_maps.append(im)

    from concourse.bass_utils import run_bass_kernel_spmd
    if PROFILE:
        _try_install_ntff_hook()
        res = run_bass_kernel_spmd(nc, in_maps, list(range(NCORES)),
                                   trace=True)
        LAST_EXEC_NS = res.exec_time_ns
        globals()["LAST_RESULTS"] = res
    else:
        res = run_bass_kernel_spmd(nc, in_maps, list(range(NCORES)))

    out = np.empty((B, S, D), np.float32)
    for c in range(NCORES):
        o = res.results[c]["out"]
        for b in range(B):
            out[b, :, 128 * c:128 * (c + 1)] = o[:, b * S:(b + 1) * S].T
    out += bof[None, None, :]
    return out



# revision 3
# speedup vs baseline: 1.2505x; 1.2505x over previous
"""DifferentialAttentionBlock on 8 NeuronCores — v2.

Sharding: DP on batch (cores 0-3 = batch 0, 4-7 = batch 1) x TP on heads
(4 heads per core).  Attention output shards are AllGathered within each
4-core batch group (not across all 8), per sq-chunk of 512, so the
collective and the Wo matmuls overlap with attention compute of the next
chunk.  Each core then computes 256 output columns of its own batch.

Changes vs v1:
  - all inputs host-pre-arranged to partition-major [128, X] layouts so
    every load DMA is a plain wide 2-D copy (no 256B-descriptor sprays)
  - loads reordered/split so the q projection starts ~2us in
  - softmax normalization: DVE reciprocal_approx_fast on the colsum row
    + gpsimd partition_broadcast + one fused scalar_tensor_tensor for
    the (a1 - lam*a2) combine; no DRAM round trips, no big DVE
    reciprocals
  - exp activations pair two sk-tiles per instruction ([128,1024])
  - per-chunk 4-rank AllGather + Wo, overlapped with attention
"""

import math
import numpy as np

B, S, D = 2, 1024, 1024
H = 16
DH = 32          # q/k half head dim
DK = 64          # v head dim
HPC = 4          # heads per core
NCORES = 8
LAMBDA_INIT = 0.8 - 0.6 * math.exp(-0.3 * (1 - 1))
NSK = S // 128   # 8 s_k tiles
CHW = 512        # sq chunk width
NCH = S // CHW   # 2 chunks
RG8 = [list(range(8))]
RG4 = [[0, 1, 2, 3], [4, 5, 6, 7]]

PROFILE = False
LAST_EXEC_NS = None
LAST_RESULTS = None

_cache = {}


def _try_install_ntff_hook():
    try:
        import sys, types
        import antenv
        try:
            import antenv.axon_hooks  # noqa: F401
            return
        except ImportError:
            pass
        mod = types.ModuleType("antenv.axon_hooks")
        mod._hook = None
        mod.set_axon_ntff_profile_hook = lambda h: setattr(mod, "_hook", h)
        mod.get_axon_ntff_profile_hook = lambda: mod._hook
        sys.modules["antenv.axon_hooks"] = mod
        antenv.axon_hooks = mod
        from trn_agent_boot.trn_boot import _ntff_profile_via_ctypes
        mod._hook = _ntff_profile_via_ctypes('/opt/axon/libaxon_pjrt.so')
    except Exception:
        pass


def _build(causal: bool):
    import concourse.bacc as bacc
    import concourse.mybir as mybir
    import concourse.tile as tile
    from concourse.tile_rust import add_dep_helper

    dt = mybir.dt
    f32, bf16 = dt.float32, dt.bfloat16
    AF = mybir.ActivationFunctionType
    OP = mybir.AluOpType

    nc = bacc.Bacc("TRN2", target_bir_lowering=False, debug=False,
                   num_devices=NCORES)

    def inp(name, shape, d=f32):
        return nc.dram_tensor(name, shape, d, kind="ExternalInput")

    # host-pre-arranged partition-major layouts
    qTh = inp("qTh", [128, 8 * S], bf16)
    kTh = inp("kTh", [128, 8 * S], bf16)
    vTh = inp("vTh", [128, 8 * S], bf16)
    Wq1 = inp("Wq1", [128, 8 * 128], bf16)
    Wq2 = inp("Wq2", [128, 8 * 128], bf16)
    Wk1 = inp("Wk1", [128, 8 * 128], bf16)
    Wk2 = inp("Wk2", [128, 8 * 128], bf16)
    Wv = inp("Wv", [128, 8 * 256], bf16)
    Wob = inp("Wob", [128, 8 * 256], bf16)      # my 256 output columns
    bq1 = inp("bq1", [128, 1]);  bq2 = inp("bq2", [128, 1])
    bk1 = inp("bk1", [128, 1]);  bk2 = inp("bk2", [128, 1])
    bv = inp("bv", [1, 256], bf16)
    ones_in = inp("ones1", [1, 128], bf16)
    triu = inp("triu", [128, 128], bf16)
    neglam = inp("neglam", [128, 1])
    maskT = None if causal else inp("maskT", [S, S])
    out_ext = nc.dram_tensor("out", [256, S], f32, kind="ExternalOutput")

    with tile.TileContext(nc) as tc:
        with (
            tc.tile_pool(name="const", bufs=1) as cpool,
            tc.tile_pool(name="wts", bufs=1) as wpool,
            tc.tile_pool(name="proj", bufs=1) as ppool,
            tc.tile_pool(name="acts", bufs=1) as apool,
            tc.tile_pool(name="edata", bufs=2) as epool,
            tc.tile_pool(name="small", bufs=2) as spool,
            tc.tile_pool(name="outs", bufs=2) as opool,
            tc.tile_pool(name="dram", bufs=1, space="DRAM") as dpool,
        ):
            # tiny AllGather issued first: absorbs cross-core launch skew
            dummy_in = dpool.tile([1, 16], bf16, name="dummy_in")
            dummy_out = dpool.tile([8, 16], bf16, name="dummy_out")
            nc.gpsimd.dma_start(dummy_in[:], triu[0:1, 0:16])
            nc.gpsimd.collective_compute(
                "AllGather", mybir.AluOpType.bypass, replica_groups=RG8,
                ins=[dummy_in.opt()], outs=[dummy_out.opt()])

            # ---- loads: q-path first so projections start ASAP ----
            wsb = {}
            for name, t in (("Wq1", Wq1), ("Wq2", Wq2),
                            ("Wk1", Wk1), ("Wk2", Wk2)):
                wsb[name] = wpool.tile([128, 8 * 128], bf16, tag=name,
                                       name=name)
                nc.scalar.dma_start(wsb[name][:], t[:, :])
            qsb = apool.tile([128, 8 * S], bf16, tag="qsb")
            ksb = apool.tile([128, 8 * S], bf16, tag="ksb")
            vsb = apool.tile([128, 8 * S], bf16, tag="vsb")
            qdma, kdma, vdma = [], [], []
            for j in range(4):
                sl = slice(2 * S * j, 2 * S * (j + 1))
                qdma.append(nc.sync.dma_start(qsb[:, sl], qTh[:, sl]))
            for j in range(4):
                sl = slice(2 * S * j, 2 * S * (j + 1))
                kdma.append(nc.sync.dma_start(ksb[:, sl], kTh[:, sl]))
            wv_sb = wpool.tile([128, 8 * 256], bf16, tag="Wv")
            nc.scalar.dma_start(wv_sb[:], Wv[:, :])
            for j in range(4):
                sl = slice(2 * S * j, 2 * S * (j + 1))
                vdma.append(nc.sync.dma_start(vsb[:, sl], vTh[:, sl]))
            wo_sb = wpool.tile([128, 8 * 256], bf16, tag="Wob")
            nc.scalar.dma_start(wo_sb[:], Wob[:, :])

            # constants on the gpsimd (SWDGE) queue
            triu_sb = cpool.tile([128, 128], bf16, tag="triu")
            nc.gpsimd.dma_start(triu_sb[:], triu[:, :])
            nlam_sb = cpool.tile([128, 1], f32, tag="neglam")
            nc.gpsimd.dma_start(nlam_sb[:], neglam[:, :])
            ones1 = cpool.tile([1, 128], bf16, tag="ones1")
            nc.gpsimd.dma_start(ones1[:], ones_in[:, :])
            bsb = {}
            for name, t in (("bq1", bq1), ("bq2", bq2), ("bk1", bk1),
                            ("bk2", bk2)):
                bsb[name] = cpool.tile([128, 1], f32, tag=name, name=name)
                nc.gpsimd.dma_start(bsb[name][:], t[:, :])
            bv_sb = cpool.tile([1, 256], bf16, tag="bv")
            nc.gpsimd.dma_start(bv_sb[:], bv[:, :])

            # kpad tiles zeroed early (DVE idle during load phase)
            kpad = {}
            for m_ in range(2):
                for h_ in range(4):
                    t_ = ppool.tile([128, S], bf16, tag=f"kp{m_}{h_}",
                                    name=f"kp{m_}{h_}")
                    nc.vector.memset(t_[:], 0.0)
                    kpad[(m_, h_)] = t_

            # ---- q projections ----
            q1T = ppool.tile([128, S], bf16, tag="q1T")
            q2T = ppool.tile([128, S], bf16, tag="q2T")
            with tc.tile_pool(name="psBq", bufs=1, space="PSUM") as psBq:
                pq1 = psBq.tile([128, S], f32, tag="q1")
                pq2 = psBq.tile([128, S], f32, tag="q2")
                for d in range(8):
                    qTd = qsb[:, d * S:(d + 1) * S]
                    for ps, wname in ((pq1, "Wq1"), (pq2, "Wq2")):
                        lhsT = wsb[wname][:, d * 128:(d + 1) * 128]
                        for half in range(2):
                            nc.tensor.matmul(
                                ps[:, half * 512:(half + 1) * 512], lhsT,
                                qTd[:, half * 512:(half + 1) * 512],
                                start=(d == 0), stop=(d == 7))
                nc.scalar.activation(q1T[:], pq1[:], AF.Identity,
                                     bias=bsb["bq1"][:])
                nc.vector.tensor_scalar(q2T[:], pq2[:], bsb["bq2"][:],
                                        None, OP.add)
            # ---- k projections (into zero-padded per-head tiles) ----
            with tc.tile_pool(name="psBk", bufs=1, space="PSUM") as psBk:
                pk1 = psBk.tile([128, S], f32, tag="k1")
                pk2 = psBk.tile([128, S], f32, tag="k2")
                for d in range(8):
                    kTd = ksb[:, d * S:(d + 1) * S]
                    for ps, wname in ((pk1, "Wk1"), (pk2, "Wk2")):
                        lhsT = wsb[wname][:, d * 128:(d + 1) * 128]
                        for half in range(2):
                            nc.tensor.matmul(
                                ps[:, half * 512:(half + 1) * 512], lhsT,
                                kTd[:, half * 512:(half + 1) * 512],
                                start=(d == 0), stop=(d == 7))
                for m, pk, bn in ((0, pk1, "bk1"), (1, pk2, "bk2")):
                    for h in range(4):
                        t = kpad[(m, h)]
                        sl = slice(32 * h, 32 * h + 32)
                        if m == 0:
                            nc.scalar.activation(t[sl, :], pk[sl, :],
                                                 AF.Identity,
                                                 bias=bsb[bn][sl, :])
                        else:
                            nc.vector.tensor_scalar(t[sl, :], pk[sl, :],
                                                    bsb[bn][sl, :],
                                                    None, OP.add)
            # ---- vv projection; vvo block i: 4 heads x (vv x64, 1) ----
            vvo = ppool.tile([128, 8 * 260], bf16, tag="vvo")
            with tc.tile_pool(name="psC", bufs=1, space="PSUM") as psC:
                pvv = [psC.tile([128, 256], f32, tag=f"vv{i}", name=f"vv{i}")
                       for i in range(8)]
                for d in range(8):
                    vTd = vsb[:, d * S:(d + 1) * S]
                    for i in range(8):
                        nc.tensor.matmul(
                            pvv[i][:], vTd[:, i * 128:(i + 1) * 128],
                            wv_sb[:, d * 256:(d + 1) * 256],
                            start=(d == 0), stop=False)
                for i in range(8):
                    nc.tensor.matmul(pvv[i][:], ones1[:], bv_sb[:],
                                     start=False, stop=True)
                    blk = vvo[:, i * 260:(i + 1) * 260]
                    blk3 = blk.rearrange("p (h c) -> p h c", c=65)
                    nc.vector.tensor_copy(
                        blk3[:, :, 0:64],
                        pvv[i].rearrange("p (h c) -> p h c", c=64))
                    nc.vector.memset(blk3[:, :, 64:65], 1.0)

            # ---- attention + per-chunk AllGather + Wo ----
            qproj = (q1T, q2T)
            with (
                tc.tile_pool(name="psS", bufs=1, space="PSUM") as psS,
                tc.tile_pool(name="psO", bufs=1, space="PSUM") as psO,
                tc.tile_pool(name="psW", bufs=2, space="PSUM") as psW,
                tc.tile_pool(name="mloc", bufs=2) as mpool,
            ):
                for c in range(NCH):
                    cs = c * CHW
                    nvalid = min(NSK, (cs + CHW) // 128) if causal else NSK
                    ilist = list(range(nvalid))
                    pairs = [(ilist[x], ilist[x + 1])
                             for x in range(0, nvalid, 2)]
                    if not causal:
                        mT = {}
                        for i in ilist:
                            mT[i] = mpool.tile([128, CHW], f32,
                                               tag=f"mT{i%2}",
                                               name=f"mT{c}{i}")
                            nc.sync.dma_start(
                                mT[i][:],
                                maskT[i * 128:(i + 1) * 128, cs:cs + CHW])
                    aT_sb = [opool.tile([128, CHW], bf16, tag=f"aT{kk}",
                                        name=f"aT{kk}c{c}")
                             for kk in range(2)]
                    last_sub = None
                    for h in range(4):
                        tfs = {}
                        for m in range(2):
                            etiles = []
                            for (i0, i1) in pairs:
                                ps = psS.tile([128, 2 * CHW], f32, tag="s",
                                              name=f"s{c}{m}{h}p{i0}",
                                              bufs=2)
                                for z, i in enumerate((i0, i1)):
                                    lo = max(cs, 128 * i) if causal else cs
                                    off = z * CHW + (lo - cs)
                                    n = cs + CHW - lo
                                    nc.tensor.matmul(
                                        ps[:, off:off + n],
                                        kpad[(m, h)][:,
                                                     i * 128:(i + 1) * 128],
                                        qproj[m][:, lo:cs + CHW],
                                        start=True, stop=True)
                                    if not causal:
                                        nc.vector.tensor_tensor(
                                            ps[:, off:off + n],
                                            ps[:, off:off + n],
                                            mT[i][:, lo - cs:CHW], OP.add)
                                e = epool.tile([128, 2 * CHW], bf16,
                                               tag=f"e{i0%4}",
                                               name=f"e{m}h{h}p{i0}c{c}")
                                nc.scalar.activation(e[:], ps[:], AF.Exp,
                                                     scale=0.125)
                                for z, i in enumerate((i0, i1)):
                                    if causal and 128 * i >= cs:
                                        doff = z * CHW + 128 * i - cs
                                        nc.vector.tensor_tensor(
                                            e[:, doff:doff + 128],
                                            e[:, doff:doff + 128],
                                            triu_sb[:], OP.mult)
                                etiles.append(((i0, i1), e))
                            # A@V in outT form; fused colsum row at 64
                            o = psO.tile([128, CHW], f32, tag=f"o{m}",
                                         name=f"o{m}h{h}c{c}")
                            x = 0
                            nmm = 2 * len(pairs)
                            for (i0, i1), e in etiles:
                                for z, i in enumerate((i0, i1)):
                                    lo = max(cs, 128 * i) if causal else cs
                                    nc.tensor.matmul(
                                        o[0:65, lo - cs:CHW],
                                        vvo[:, 260 * i + 65 * h:
                                            260 * i + 65 * h + 65],
                                        e[:, z * CHW + lo - cs:
                                          (z + 1) * CHW],
                                        start=(x == 0), stop=(x == nmm - 1))
                                    x += 1
                            # 1/colsum on the row, broadcast, scale out
                            rcw = spool.tile([1, CHW], f32, tag=f"rw{m}",
                                             name=f"rw{m}h{h}c{c}")
                            nc.vector.tensor_copy(rcw[:], o[64:65, 0:CHW])
                            rc = spool.tile([1, CHW], f32, tag=f"rc{m}",
                                            name=f"rc{m}h{h}c{c}")
                            nc.vector.reciprocal_approx_fast(rc[:], rcw[:])
                            rb = spool.tile([64, CHW], f32, tag=f"rb{m}",
                                            name=f"rb{m}h{h}c{c}")
                            nc.gpsimd.partition_broadcast(rb[:], rc[:])
                            tf = spool.tile([64, CHW], f32, tag=f"tf{m}",
                                            name=f"tf{m}h{h}c{c}")
                            nc.vector.tensor_tensor(tf[:], o[0:64, 0:CHW],
                                                    rb[:], OP.mult)
                            tfs[m] = tf
                        dst = aT_sb[h // 2][64 * (h % 2):64 * (h % 2) + 64,
                                            :]
                        last_sub = nc.vector.scalar_tensor_tensor(
                            dst, tfs[1][:], nlam_sb[0:64, 0:1], tfs[0][:],
                            OP.mult, OP.add)
                    # ship this chunk: batch-group AllGather + Wo shard
                    bounce = dpool.tile([256, CHW], bf16, name=f"bnc{c}")
                    for kk in range(2):
                        nc.sync.dma_start(
                            bounce[128 * kk:128 * (kk + 1), :], aT_sb[kk][:])
                    ag = dpool.tile([4 * 256, CHW], bf16, name=f"ag{c}")
                    agop = nc.gpsimd.collective_compute(
                        "AllGather", mybir.AluOpType.bypass,
                        replica_groups=RG4,
                        ins=[bounce.opt()], outs=[ag.opt()])
                    myt = mpool.tile([128, 8 * CHW], bf16, tag="myt",
                                     name=f"myt{c}", bufs=2)
                    mydma = nc.sync.dma_start(
                        myt[:].rearrange("p (k x) -> p k x", k=8),
                        ag.rearrange("(k p) x -> p k x", p=128))
                    add_dep_helper(mydma.ins, last_sub.ins,
                                   reason="wo after local normalize")
                    for cg in range(2):
                        wps = psW.tile([128, CHW], f32, tag="wo",
                                       name=f"wo{c}{cg}")
                        for k in range(8):
                            nc.tensor.matmul(
                                wps[:],
                                wo_sb[:, 256 * k + 128 * cg:
                                      256 * k + 128 * (cg + 1)],
                                myt[:, CHW * k:CHW * (k + 1)],
                                start=(k == 0), stop=(k == 7))
                        osb = opool.tile([128, CHW], f32, tag="osb",
                                         name=f"osb{c}{cg}")
                        if cg == 0:
                            nc.scalar.copy(osb[:], wps[:])
                        else:
                            nc.vector.tensor_copy(osb[:], wps[:])
                        nc.scalar.dma_start(
                            out_ext[128 * cg:128 * (cg + 1),
                                    cs:cs + CHW], osb[:])

    nc.compile()
    return nc


def kernel(**inputs):
    global LAST_EXEC_NS
    import ml_dtypes

    q = np.asarray(inputs["q"], dtype=np.float32)
    k = np.asarray(inputs["k"], dtype=np.float32)
    v = np.asarray(inputs["v"], dtype=np.float32)
    mask = np.asarray(inputs["mask"])
    f32 = np.float32
    Wq1f = np.asarray(inputs["Wq1"], f32); Wq2f = np.asarray(inputs["Wq2"], f32)
    Wk1f = np.asarray(inputs["Wk1"], f32); Wk2f = np.asarray(inputs["Wk2"], f32)
    Wvf = np.asarray(inputs["Wv"], f32);   Wof = np.asarray(inputs["Wo"], f32)
    bq1f = np.asarray(inputs["bq1"], f32); bq2f = np.asarray(inputs["bq2"], f32)
    bk1f = np.asarray(inputs["bk1"], f32); bk2f = np.asarray(inputs["bk2"], f32)
    bvf = np.asarray(inputs["bv"], f32);   bof = np.asarray(inputs["bo"], f32)
    lam = float(np.exp(float(inputs["lq1"][0]) * float(inputs["lk1"][0]))
                - np.exp(float(inputs["lq2"][0]) * float(inputs["lk2"][0]))
                + LAMBDA_INIT)

    mk = (mask.reshape(B, S, S) != 0)
    causal = bool((mk == np.tril(np.ones((S, S), bool))[None]).all())

    key = "causal" if causal else "general"
    if key not in _cache:
        _cache[key] = _build(causal)
    nc = _cache[key]

    bfl = ml_dtypes.bfloat16

    def pmaj(x, width):
        # [1024, width] -> [128, 8*width] partition-major over 8 d-tiles
        return np.ascontiguousarray(
            x.reshape(8, 128, width).transpose(1, 0, 2).reshape(
                128, 8 * width)).astype(bfl)

    qTl = [pmaj(q[b].T, S) for b in range(B)]
    kTl = [pmaj(k[b].T, S) for b in range(B)]
    vTl = [pmaj(v[b].T, S) for b in range(B)]
    triu = np.triu(np.ones((128, 128))).astype(bfl)
    nlam = np.full((128, 1), -lam, f32)
    maskTs = None
    if not causal:
        maskTs = [np.ascontiguousarray(
            np.where(mk[b], np.float32(0), np.float32(-1e9)).T)
            for b in range(B)]

    in_maps = []
    for c in range(NCORES):
        b, g = divmod(c, 4)
        im = dict(
            qTh=qTl[b], kTh=kTl[b], vTh=vTl[b],
            Wq1=pmaj(Wq1f[:, 128 * g:128 * (g + 1)], 128),
            Wq2=pmaj(Wq2f[:, 128 * g:128 * (g + 1)], 128),
            Wk1=pmaj(Wk1f[:, 128 * g:128 * (g + 1)], 128),
            Wk2=pmaj(Wk2f[:, 128 * g:128 * (g + 1)], 128),
            Wv=pmaj(Wvf[:, 256 * g:256 * (g + 1)], 256),
            Wob=pmaj(Wof[:, 256 * g:256 * (g + 1)], 256),
            bq1=np.ascontiguousarray(bq1f[128 * g:128 * (g + 1)]).reshape(128, 1),
            bq2=np.ascontiguousarray(bq2f[128 * g:128 * (g + 1)]).reshape(128, 1),
            bk1=np.ascontiguousarray(bk1f[128 * g:128 * (g + 1)]).reshape(128, 1),
            bk2=np.ascontiguousarray(bk2f[128 * g:128 * (g + 1)]).reshape(128, 1),
            bv=np.ascontiguousarray(bvf[256 * g:256 * (g + 1)]).reshape(1, 256).astype(bfl),
            triu=triu, neglam=nlam,
            ones1=np.ones((1, 128), bfl),
        )
        if not causal:
            im["maskT"] = maskTs[b]
        in_maps.append(im)

    from concourse.bass_utils import run_bass_kernel_spmd
    if PROFILE:
        _try_install_ntff_hook()
        res = run_bass_kernel_spmd(nc, in_maps, list(range(NCORES)),
                                   trace=True)
        LAST_EXEC_NS = res.exec_time_ns
        globals()["LAST_RESULTS"] = res
    else:
        res = run_bass_kernel_spmd(nc, in_maps, list(range(NCORES)))

    out = np.empty((B, S, D), np.float32)
    for c in range(NCORES):
        b, g = divmod(c, 4)
        o = res.results[c]["out"]          # [256 cols, 1024 pos]
        out[b, :, 256 * g:256 * (g + 1)] = o.T
    out += bof[None, None, :]
    return out
